# revision 26
# baseline (speedup 1.0000x reference)
"""DeepSeekMoE Trainium2 kernel (8 NeuronCores, data-parallel over tokens).

Reference computation (B=128, FEW=64, D=512, E=16, O=512, H=64, K=3):
  t = x.reshape(T=8192, D)
  gates = softmax(relu(t@gW1+gb1)@gW2+gb2)            # [T, E]
  h  = relu(einsum('td,edh->teh', t, W1) + b1)        # [T, E, H]
  eo = einsum('teh,eho->teo', h, W2) + b2             # [T, E, O]
  topv, topi = top_k(gates, 3); out_t = sum_k topv * eo[topi]
  out = mean over FEW  -> [B, 1, 1, O]

Kernel restructure (per core, 1024 tokens = 16 B-rows):
  gmask[t,e] = gates[t,e] if gates[t,e] in top-3 of row t else 0
  out_t      = sum_e gmask[t,e] * (relu(x_t W1_e + b1_e) W2_e) + gmask @ b2
and the FEW-mean is folded into W2/b2 (scaled by 1/64) with a final
block-summing matmul.  All heavy matmuls run as float32r (full-rate PE).

Experts are processed in pairs stacked along the 128-partition dim
(h2 = s*64 + h, e = 2*pair + s).  Gate broadcast across the 64 h-rows is
done with a tiny constant "mask matmul" on the PE.
"""

import sys

import numpy as np

for _p in ("/opt/trn_rl_repo",):
    if _p not in sys.path:
        sys.path.insert(0, _p)

B, FEW, D = 128, 64, 512
E, O, H, TOPK = 16, 512, 64, 3
T = B * FEW            # 8192 tokens
NCORES = 8
TLOC = T // NCORES     # 1024 tokens per core
DT = 256               # tokens per "double tile" (fp32r wants moving dim >= 256)
NDT = TLOC // DT       # 4 double tiles per core
PAIRS = E // 2         # 8 expert pairs
NSLICE = TLOC // 128   # 8 token slices of 128 per core

_CACHE = {}


def _build_nc():
    import concourse.mybir as mybir
    import concourse.tile as tile
    from concourse import bacc

    f32 = mybir.dt.float32
    f32r = mybir.dt.float32r
    f16 = mybir.dt.float16
    AF = mybir.ActivationFunctionType
    ALU = mybir.AluOpType
    AX = mybir.AxisListType

    nc = bacc.Bacc("TRN2", target_bir_lowering=False, debug=False,
                   num_devices=NCORES)

    # ---- DRAM I/O ----------------------------------------------------------
    xt16_d = nc.dram_tensor("xt16", [D, TLOC], f16, kind="ExternalInput")
    xlo_d = nc.dram_tensor("xlo", [D, TLOC], f16, kind="ExternalInput")
    w1_d = nc.dram_tensor("w1", [128, 4, PAIRS, 128], f16, kind="ExternalInput")
    w2_d = nc.dram_tensor("w2", [128, PAIRS, O], f16, kind="ExternalInput")
    b1_d = nc.dram_tensor("b1", [128, PAIRS], f32, kind="ExternalInput")
    b2_d = nc.dram_tensor("b2", [E, O], f16, kind="ExternalInput")
    gw1_d = nc.dram_tensor("gw1", [128, 4, 2, H], f16, kind="ExternalInput")
    gb1_d = nc.dram_tensor("gb1", [H, 1], f32, kind="ExternalInput")
    gw2a_d = nc.dram_tensor("gw2a", [H + 1, E], f32, kind="ExternalInput")
    ident_d = nc.dram_tensor("ident", [128, 128], f32, kind="ExternalInput")
    maskp_d = nc.dram_tensor("maskp", [E, PAIRS, 128], f16, kind="ExternalInput")
    meanm_d = nc.dram_tensor("meanm", [128, NSLICE, E], f16, kind="ExternalInput")
    out_d = nc.dram_tensor("out", [16, O], f32, kind="ExternalOutput")

    with tile.TileContext(nc) as tc:
        with (
            tc.tile_pool(name="consts", bufs=1) as consts,
            tc.tile_pool(name="work", bufs=3) as work,
            tc.tile_pool(name="psH", bufs=2, space="PSUM") as psH,
            tc.tile_pool(name="psOut", bufs=2, space="PSUM") as psOut,
            tc.tile_pool(name="psSmall", bufs=2, space="PSUM") as psSmall,
            tc.tile_pool(name="psGbc", bufs=2, space="PSUM") as psGbc,
        ):
            # ---- resident SBUF loads --------------------------------------
            # Two HWDGE rings (SP + ACT); gating-critical tensors first on
            # each so the first matmuls start early.  xt/w1 split per d-chunk.
            xt16sb = consts.tile([128, 4, TLOC], f16)
            xlosb = consts.tile([128, 4, TLOC], f16)
            xlo_r = xlo_d.ap().rearrange("(j p) t -> p j t", p=128)
            gw1sb = consts.tile([128, 4, 2, H], f16)
            w1sb = consts.tile([128, 4, PAIRS, 128], f16)
            xt16_r = xt16_d.ap().rearrange("(j p) t -> p j t", p=128)
            gb1sb = consts.tile([H, 1], f32)
            b1sb = consts.tile([128, PAIRS], f32)
            # fp16 expert inputs first (MM1 work starts earliest), gating fp32
            # x after, weights/consts interleaved by first-use time.
            nc.sync.dma_start(out=gw1sb, in_=gw1_d.ap())
            nc.sync.dma_start(out=xt16sb[:, 0, :], in_=xt16_r[:, 0, :])
            nc.sync.dma_start(out=w1sb[:, 0, :, :], in_=w1_d.ap()[:, 0, :, :])
            nc.sync.dma_start(out=gb1sb, in_=gb1_d.ap())
            nc.sync.dma_start(out=b1sb, in_=b1_d.ap())
            for j in range(1, 4):
                nc.sync.dma_start(out=xt16sb[:, j, :], in_=xt16_r[:, j, :])
                nc.sync.dma_start(out=w1sb[:, j, :, :], in_=w1_d.ap()[:, j, :, :])
            identsb = consts.tile([128, 128], f32)
            nc.sync.dma_start(out=identsb, in_=ident_d.ap())
            maskpsb = consts.tile([E, PAIRS, 128], f16)
            nc.sync.dma_start(out=maskpsb, in_=maskp_d.ap())
            gw2asb = consts.tile([H + 1, E], f32)
            nc.sync.dma_start(out=gw2asb, in_=gw2a_d.ap())
            nc.sync.dma_start(out=xlosb, in_=xlo_r)
            b2sb = consts.tile([E, O], f16)
            nc.sync.dma_start(out=b2sb, in_=b2_d.ap())
            w2sb = consts.tile([128, PAIRS, O], f16)
            for pr in range(PAIRS):
                nc.sync.dma_start(out=w2sb[:, pr, :], in_=w2_d.ap()[:, pr, :])
            meanmsb = consts.tile([128, NSLICE, E], f16)
            nc.sync.dma_start(out=meanmsb, in_=meanm_d.ap())


            # gating hidden activations for the whole core, row H is the
            # constant 1.0 row that folds gb2 into the logits matmul
            asb = consts.tile([H + 1, TLOC], f32)
            nc.scalar.activation(asb[H:H + 1, :], asb[H:H + 1, :], AF.Copy,
                                 bias=1.0, scale=0.0)

            def mm1_pair(pair, t0):
                """MM1 + relu for one expert pair; returns hr tile."""
                psh = psH.tile([128, DT], f32, tag="psh",
                               name=f"psh_{t0}_{pair}")
                for j in range(4):
                    nc.tensor.matmul(psh, w1sb[:, j, pair, :],
                                     xt16sb[:, j, t0:t0 + DT],
                                     start=(j == 0), stop=(j == 3))
                hr = work.tile([128, DT], f16, tag="hr", bufs=9,
                               name=f"hr_{t0}_{pair}")
                nc.scalar.activation(hr, psh, AF.Relu,
                                     bias=b1sb[:, pair:pair + 1], scale=1.0)
                return hr

            for dti in range(NDT):
                t0 = dti * DT
                # ---- gating hidden: A^T = relu(gW1^T X^T + gb1) -----------
                # 3-pass fp16 hi/lo decomposition, exact to ~1e-7:
                #   A = xh@gh + (xh@gl + xl@gh) / 2048
                psA = psSmall.tile([H, DT], f32, tag="small")
                psA2 = psSmall.tile([H, DT], f32, tag="small")
                for j in range(4):
                    nc.tensor.matmul(psA, gw1sb[:, j, 0, :],
                                     xt16sb[:, j, t0:t0 + DT],
                                     start=(j == 0), stop=(j == 3))
                    nc.tensor.matmul(psA2, gw1sb[:, j, 1, :],
                                     xt16sb[:, j, t0:t0 + DT],
                                     start=(j == 0), stop=False)
                    nc.tensor.matmul(psA2, gw1sb[:, j, 0, :],
                                     xlosb[:, j, t0:t0 + DT],
                                     start=False, stop=(j == 3))
                sbA2 = work.tile([H, DT], f32, tag="sbA2")
                nc.vector.tensor_copy(sbA2, psA2)
                zsb = work.tile([H, DT], f32, tag="zsb")
                nc.vector.scalar_tensor_tensor(zsb, sbA2, 1.0 / 2048.0, psA,
                                               op0=ALU.mult, op1=ALU.add)
                nc.scalar.activation(asb[0:H, t0:t0 + DT], zsb, AF.Relu,
                                     bias=gb1sb, scale=1.0)
                # first half of the expert MM1s keeps PE busy while the
                # gating softmax chain runs on DVE/ACT
                hrs = [mm1_pair(pair, t0) for pair in range(4)]
                # ---- logits + softmax + top-3 mask + transpose ------------
                gmt = work.tile([E, DT], f16, tag="gmt")
                for s in range(2):
                    st = s * 128
                    psL = psSmall.tile([128, E], f32, tag="small")
                    nc.tensor.matmul(psL, asb[:, t0 + st:t0 + st + 128], gw2asb)
                    negmax = work.tile([128, 1], f32, tag="negmax")
                    nc.vector.tensor_reduce(negmax, psL, axis=AX.X, op=ALU.max,
                                            negate=True)
                    expd = work.tile([128, E], f32, tag="expd")
                    sume = work.tile([128, 1], f32, tag="sume")
                    nc.scalar.activation(expd, psL, AF.Exp, bias=negmax,
                                         scale=1.0, accum_out=sume)
                    rsum = work.tile([128, 1], f32, tag="rsum")
                    nc.vector.reciprocal(rsum, sume)
                    gfull = work.tile([128, E], f32, tag="gfull")
                    nc.vector.tensor_scalar_mul(gfull, expd, rsum)
                    top8 = work.tile([128, 8], f32, tag="top8")
                    nc.vector.max(top8, gfull)
                    gmask = work.tile([128, E], f32, tag="gmask")
                    nc.vector.scalar_tensor_tensor(gmask, gfull, top8[:, 2:3],
                                                   gfull, op0=ALU.is_ge,
                                                   op1=ALU.mult)
                    psGT = psSmall.tile([E, 128], f32, tag="small")
                    nc.tensor.transpose(psGT, gmask, identsb)
                    nc.scalar.copy(gmt[:, st:st + 128], psGT)
                hrs += [mm1_pair(pair, t0) for pair in range(4, PAIRS)]

                # ---- gate-scale + second expert matmul --------------------
                psO = [psOut.tile([128, O], f32, tag="psO", name=f"psO{dti}_{s}")
                       for s in range(2)]
                for pair in range(PAIRS):
                    psG = psGbc.tile([128, DT], f32, tag="psG", name=f"psG{dti}_{pair}")
                    nc.tensor.matmul(psG, maskpsb[:, pair, :], gmt)
                    hg = work.tile([128, DT], f16, tag="hg")
                    nc.vector.tensor_mul(hg, hrs[pair], psG)
                    for s in range(2):
                        st = s * 128
                        nc.tensor.matmul(psO[s], hg[:, st:st + 128],
                                         w2sb[:, pair, :],
                                         start=(pair == 0), stop=False)
                outs = []
                for s in range(2):
                    st = s * 128
                    nc.tensor.matmul(psO[s], gmt[:, st:st + 128], b2sb,
                                     start=False, stop=True)
                for s in range(2):
                    outsb = work.tile([128, O], f16, tag="outsb",
                                      name=f"outsb{dti}_{s}")
                    if s == 0:
                        nc.vector.tensor_copy(outsb, psO[s])
                    else:
                        nc.scalar.copy(outsb, psO[s])
                    outs.append(outsb)
                for s in range(2):
                    s8 = dti * 2 + s
                    psM = psGbc.tile([16, O], f32, tag="psG",
                                     name=f"psM{dti}_{s}")
                    nc.tensor.matmul(psM, meanmsb[:, s8, :], outs[s])
                    r0 = 2 * s8
                    rowsb = work.tile([16, O], f32, tag="rowsb",
                                      name=f"rowsb{dti}_{s}")
                    nc.vector.tensor_copy(rowsb, psM)
                    nc.sync.dma_start(out=out_d.ap()[r0:r0 + 2, :],
                                      in_=rowsb[r0:r0 + 2, :])


    nc.compile()
    return nc


def _host_inputs(x, gW1, gb1, gW2, gb2, W1, b1, W2, b2):
    """Per-core in_maps with all host-side layout transforms."""
    f = np.float32
    xt_full = np.ascontiguousarray(x.reshape(T, D).T.astype(f))       # [D, T]
    # W1 [E,D,H] -> [p, j, pair, s*64+h], e = 2*pair+s, d = 128*j+p
    w1sb = np.ascontiguousarray(
        W1.reshape(PAIRS, 2, 4, 128, H).transpose(3, 2, 0, 1, 4)
        .reshape(128, 4, PAIRS, 128).astype(f))
    # W2 [E,H,O] -> [s*64+h, pair, o], mean folded
    w2sb = np.ascontiguousarray(
        W2.reshape(PAIRS, 2, H, O).transpose(1, 2, 0, 3)
        .reshape(128, PAIRS, O).astype(f) / np.float32(FEW))
    b1sb = np.ascontiguousarray(
        b1.reshape(PAIRS, 2, H).transpose(1, 2, 0).reshape(128, PAIRS).astype(f))
    b2sb = np.ascontiguousarray(b2.astype(f) / np.float32(FEW))
    gw1f = gW1.reshape(4, 128, H).transpose(1, 0, 2).astype(f)  # [128,4,64]
    gw1hi = gw1f.astype(np.float16)
    gw1lo = ((gw1f - gw1hi.astype(f)) * 2048.0).astype(np.float16)
    gw1a = np.ascontiguousarray(
        np.stack([gw1hi, gw1lo], axis=2))                        # [128,4,2,64]
    gb1sb = np.ascontiguousarray(gb1.reshape(H, 1).astype(f))
    gw2a = np.ascontiguousarray(
        np.vstack([gW2.astype(f), gb2.reshape(1, E).astype(f)]))
    ident = np.eye(128, dtype=f)
    # maskp[e, pair, m] = 1 if e == 2*pair + m//64
    m = np.arange(128)
    pr = np.arange(PAIRS)
    ee = np.arange(E)
    maskp = (ee[:, None, None] == (2 * pr[None, :, None] + m[None, None, :] // 64)
             ).astype(f)
    # meanm[p, s, rrow] = 1 if rrow == 2*s + p//64
    ss = np.arange(NSLICE)
    rr = np.arange(E)
    meanm = (rr[None, None, :] == (2 * ss[None, :, None] + m[:, None, None] // 64)
             ).astype(f)

    h = np.float16
    shared = dict(w1=w1sb.astype(h), w2=w2sb.astype(h), b1=b1sb,
                  b2=b2sb.astype(h), gw1=gw1a, gb1=gb1sb, gw2a=gw2a,
                  ident=ident, maskp=maskp.astype(h), meanm=meanm.astype(h))
    in_maps = []
    for c in range(NCORES):
        im = dict(shared)
        xt_c = np.ascontiguousarray(xt_full[:, c * TLOC:(c + 1) * TLOC])
        xhi = xt_c.astype(h)
        im["xt16"] = xhi
        im["xlo"] = ((xt_c - xhi.astype(f)) * 2048.0).astype(h)
        in_maps.append(im)
    return in_maps


def kernel(x, gW1, gb1, gW2, gb2, W1, b1, W2, b2, _trace=False):
    from concourse.bass_utils import run_bass_kernel_spmd

    if "nc" not in _CACHE:
        _CACHE["nc"] = _build_nc()
    nc = _CACHE["nc"]
    in_maps = _host_inputs(x, gW1, gb1, gW2, gb2, W1, b1, W2, b2)
    try:
        kres = run_bass_kernel_spmd(nc, in_maps, core_ids=list(range(NCORES)),
                                    trace=_trace)
    except ModuleNotFoundError:
        # NTFF profile hook absent in this container; run without trace
        kres = run_bass_kernel_spmd(nc, in_maps, core_ids=list(range(NCORES)),
                                    trace=False)
    _CACHE["last_result"] = kres
    out = np.concatenate([kres.results[c]["out"] for c in range(NCORES)], axis=0)
    return out.reshape(B, 1, 1, O).astype(np.float32)


# revision 31
# speedup vs baseline: 1.0199x; 1.0199x over previous
"""DeepSeekMoE Trainium2 kernel (8 NeuronCores, data-parallel over tokens).

Reference computation (B=128, FEW=64, D=512, E=16, O=512, H=64, K=3):
  t = x.reshape(T=8192, D)
  gates = softmax(relu(t@gW1+gb1)@gW2+gb2)            # [T, E]
  h  = relu(einsum('td,edh->teh', t, W1) + b1)        # [T, E, H]
  eo = einsum('teh,eho->teo', h, W2) + b2             # [T, E, O]
  topv, topi = top_k(gates, 3); out_t = sum_k topv * eo[topi]
  out = mean over FEW  -> [B, 1, 1, O]

Kernel restructure (per core, 1024 tokens = 16 B-rows):
  gmask[t,e] = gates[t,e] if gates[t,e] in top-3 of row t else 0
  out_t      = sum_e gmask[t,e] * (relu(x_t W1_e + b1_e) W2_e) + gmask @ b2
and the FEW-mean is folded into W2/b2 (scaled by 1/64) with final
block-summing matmuls.

Precision: the expert path (MM1/MM2 and gate application) runs in fp16
with fp32 PSUM accumulation (~2.5e-4 rel error).  The gating network,
which must reproduce the reference's top-3 SELECTION exactly, runs the
hidden layer as a 3-pass fp16 hi/lo split (x = xh + xlo/2048,
gW1 = gh + gl/2048 -> xh@gh + (xh@gl + xlo@gh)/2048, accurate to ~1e-7)
and the tiny logits matmul in fp32, so no selection flips occur.

Experts are processed in pairs stacked along the 128-partition dim
(h2 = s*64 + h, e = 2*pair + s).  The per-pair gate broadcast across the
64 h-rows is a tiny constant "mask matmul" on the PE; gb2 is folded into
the logits matmul via a constant-1 row of the activation tile.
"""

import sys

import numpy as np

for _p in ("/opt/trn_rl_repo",):
    if _p not in sys.path:
        sys.path.insert(0, _p)

B, FEW, D = 128, 64, 512
E, O, H, TOPK = 16, 512, 64, 3
T = B * FEW            # 8192 tokens
NCORES = 8
TLOC = T // NCORES     # 1024 tokens per core
DT = 256               # tokens per "double tile" (fp32r wants moving dim >= 256)
NDT = TLOC // DT       # 4 double tiles per core
PAIRS = E // 2         # 8 expert pairs
NSLICE = TLOC // 128   # 8 token slices of 128 per core

_CACHE = {}


def _build_nc():
    import concourse.mybir as mybir
    import concourse.tile as tile
    from concourse import bacc

    f32 = mybir.dt.float32
    f32r = mybir.dt.float32r
    f16 = mybir.dt.float16
    AF = mybir.ActivationFunctionType
    ALU = mybir.AluOpType
    AX = mybir.AxisListType

    nc = bacc.Bacc("TRN2", target_bir_lowering=False, debug=False,
                   num_devices=NCORES)

    # ---- DRAM I/O ----------------------------------------------------------
    xt16_d = nc.dram_tensor("xt16", [D, TLOC], f16, kind="ExternalInput")
    xlo_d = nc.dram_tensor("xlo", [D, TLOC], f16, kind="ExternalInput")
    w1_d = nc.dram_tensor("w1", [128, 4, PAIRS, 128], f16, kind="ExternalInput")
    w2_d = nc.dram_tensor("w2", [128, PAIRS, O], f16, kind="ExternalInput")
    b1_d = nc.dram_tensor("b1", [128, PAIRS], f32, kind="ExternalInput")
    b2_d = nc.dram_tensor("b2", [E, O], f16, kind="ExternalInput")
    gw1_d = nc.dram_tensor("gw1", [128, 4, 2, H], f16, kind="ExternalInput")
    gb1_d = nc.dram_tensor("gb1", [H, 1], f32, kind="ExternalInput")
    gw2a_d = nc.dram_tensor("gw2a", [H + 1, E], f32, kind="ExternalInput")
    ident_d = nc.dram_tensor("ident", [128, 128], f32, kind="ExternalInput")
    maskp_d = nc.dram_tensor("maskp", [E, PAIRS, 128], f16, kind="ExternalInput")
    meanm_d = nc.dram_tensor("meanm", [128, NSLICE, E], f16, kind="ExternalInput")
    out_d = nc.dram_tensor("out", [16, O], f32, kind="ExternalOutput")

    with tile.TileContext(nc) as tc:
        with (
            tc.tile_pool(name="consts", bufs=1) as consts,
            tc.tile_pool(name="work", bufs=3) as work,
            tc.tile_pool(name="psH", bufs=2, space="PSUM") as psH,
            tc.tile_pool(name="psOut", bufs=2, space="PSUM") as psOut,
            tc.tile_pool(name="psSmall", bufs=2, space="PSUM") as psSmall,
            tc.tile_pool(name="psGbc", bufs=2, space="PSUM") as psGbc,
        ):
            # ---- resident SBUF loads --------------------------------------
            # Two HWDGE rings (SP + ACT); gating-critical tensors first on
            # each so the first matmuls start early.  xt/w1 split per d-chunk.
            xt16sb = consts.tile([128, 4, TLOC], f16)
            xlosb = consts.tile([128, 4, TLOC], f16)
            xlo_r = xlo_d.ap().rearrange("(j p) t -> p j t", p=128)
            gw1sb = consts.tile([128, 4, 2, H], f16)
            w1sb = consts.tile([128, 4, PAIRS, 128], f16)
            xt16_r = xt16_d.ap().rearrange("(j p) t -> p j t", p=128)
            gb1sb = consts.tile([H, 1], f32)
            b1sb = consts.tile([128, PAIRS], f32)
            # fp16 expert inputs first (MM1 work starts earliest), gating fp32
            # x after, weights/consts interleaved by first-use time.
            nc.sync.dma_start(out=gw1sb, in_=gw1_d.ap())
            nc.sync.dma_start(out=xt16sb[:, 0, :], in_=xt16_r[:, 0, :])
            nc.sync.dma_start(out=w1sb[:, 0, :, :], in_=w1_d.ap()[:, 0, :, :])
            nc.sync.dma_start(out=gb1sb, in_=gb1_d.ap())
            nc.sync.dma_start(out=b1sb, in_=b1_d.ap())
            for j in range(1, 4):
                nc.sync.dma_start(out=xt16sb[:, j, :], in_=xt16_r[:, j, :])
                nc.sync.dma_start(out=w1sb[:, j, :, :], in_=w1_d.ap()[:, j, :, :])
            identsb = consts.tile([128, 128], f32)
            nc.sync.dma_start(out=identsb, in_=ident_d.ap())
            maskpsb = consts.tile([E, PAIRS, 128], f16)
            nc.sync.dma_start(out=maskpsb, in_=maskp_d.ap())
            gw2asb = consts.tile([H + 1, E], f32)
            nc.sync.dma_start(out=gw2asb, in_=gw2a_d.ap())
            nc.sync.dma_start(out=xlosb, in_=xlo_r)
            b2sb = consts.tile([E, O], f16)
            nc.sync.dma_start(out=b2sb, in_=b2_d.ap())
            w2sb = consts.tile([128, PAIRS, O], f16)
            for pr in range(PAIRS):
                nc.sync.dma_start(out=w2sb[:, pr, :], in_=w2_d.ap()[:, pr, :])
            meanmsb = consts.tile([128, NSLICE, E], f16)
            nc.sync.dma_start(out=meanmsb, in_=meanm_d.ap())


            # gating hidden activations for the whole core, row H is the
            # constant 1.0 row that folds gb2 into the logits matmul
            asb = consts.tile([H + 1, TLOC], f32)
            nc.vector.memset(asb[H:H + 1, :], 1.0)

            def mm1_pair(pair, t0):
                """MM1 + relu for one expert pair; returns hr tile."""
                psh = psH.tile([128, DT], f32, tag="psh",
                               name=f"psh_{t0}_{pair}")
                for j in range(4):
                    nc.tensor.matmul(psh, w1sb[:, j, pair, :],
                                     xt16sb[:, j, t0:t0 + DT],
                                     start=(j == 0), stop=(j == 3))
                hr = work.tile([128, DT], f16, tag="hr", bufs=9,
                               name=f"hr_{t0}_{pair}")
                nc.scalar.activation(hr, psh, AF.Relu,
                                     bias=b1sb[:, pair:pair + 1], scale=1.0)
                return hr

            for dti in range(NDT):
                t0 = dti * DT
                # ---- gating hidden: A^T = relu(gW1^T X^T + gb1) -----------
                # 3-pass fp16 hi/lo decomposition, exact to ~1e-7:
                #   A = xh@gh + (xh@gl + xl@gh) / 2048
                psA = psSmall.tile([H, DT], f32, tag="small")
                psA2 = psSmall.tile([H, DT], f32, tag="small")
                for j in range(4):
                    nc.tensor.matmul(psA, gw1sb[:, j, 0, :],
                                     xt16sb[:, j, t0:t0 + DT],
                                     start=(j == 0), stop=(j == 3))
                    nc.tensor.matmul(psA2, gw1sb[:, j, 1, :],
                                     xt16sb[:, j, t0:t0 + DT],
                                     start=(j == 0), stop=False)
                    nc.tensor.matmul(psA2, gw1sb[:, j, 0, :],
                                     xlosb[:, j, t0:t0 + DT],
                                     start=False, stop=(j == 3))
                sbA2 = work.tile([H, DT], f32, tag="sbA2")
                nc.vector.tensor_copy(sbA2, psA2)
                zsb = work.tile([H, DT], f32, tag="zsb")
                nc.vector.scalar_tensor_tensor(zsb, sbA2, 1.0 / 2048.0, psA,
                                               op0=ALU.mult, op1=ALU.add)
                nc.scalar.activation(asb[0:H, t0:t0 + DT], zsb, AF.Relu,
                                     bias=gb1sb, scale=1.0)
                # first half of the expert MM1s keeps PE busy while the
                # gating softmax chain runs on DVE/ACT
                hrs = [mm1_pair(pair, t0) for pair in range(4)]
                # ---- logits + softmax + top-3 mask + transpose ------------
                gmt = work.tile([E, DT], f16, tag="gmt")
                for s in range(2):
                    st = s * 128
                    psL = psSmall.tile([128, E], f32, tag="small")
                    nc.tensor.matmul(psL, asb[:, t0 + st:t0 + st + 128], gw2asb)
                    negmax = work.tile([128, 1], f32, tag="negmax")
                    nc.vector.tensor_reduce(negmax, psL, axis=AX.X, op=ALU.max,
                                            negate=True)
                    expd = work.tile([128, E], f32, tag="expd")
                    sume = work.tile([128, 1], f32, tag="sume")
                    nc.scalar.activation(expd, psL, AF.Exp, bias=negmax,
                                         scale=1.0, accum_out=sume)
                    rsum = work.tile([128, 1], f32, tag="rsum")
                    nc.vector.reciprocal(rsum, sume)
                    gfull = work.tile([128, E], f32, tag="gfull")
                    nc.vector.tensor_scalar_mul(gfull, expd, rsum)
                    top8 = work.tile([128, 8], f32, tag="top8")
                    nc.vector.max(top8, gfull)
                    gmask = work.tile([128, E], f32, tag="gmask")
                    nc.vector.scalar_tensor_tensor(gmask, gfull, top8[:, 2:3],
                                                   gfull, op0=ALU.is_ge,
                                                   op1=ALU.mult)
                    psGT = psSmall.tile([E, 128], f32, tag="small")
                    nc.tensor.transpose(psGT, gmask, identsb)
                    nc.scalar.copy(gmt[:, st:st + 128], psGT)
                hrs += [mm1_pair(pair, t0) for pair in range(4, PAIRS)]

                # ---- gate-scale + second expert matmul --------------------
                psO = [psOut.tile([128, O], f32, tag="psO", name=f"psO{dti}_{s}")
                       for s in range(2)]
                for pair in range(PAIRS):
                    psG = psGbc.tile([128, DT], f32, tag="psG", name=f"psG{dti}_{pair}")
                    nc.tensor.matmul(psG, maskpsb[:, pair, :], gmt)
                    hg = work.tile([128, DT], f16, tag="hg")
                    nc.vector.tensor_mul(hg, hrs[pair], psG)
                    for s in range(2):
                        st = s * 128
                        nc.tensor.matmul(psO[s], hg[:, st:st + 128],
                                         w2sb[:, pair, :],
                                         start=(pair == 0), stop=False)
                outs = []
                for s in range(2):
                    st = s * 128
                    nc.tensor.matmul(psO[s], gmt[:, st:st + 128], b2sb,
                                     start=False, stop=True)
                for s in range(2):
                    outsb = work.tile([128, O], f16, tag="outsb",
                                      name=f"outsb{dti}_{s}")
                    if s == 0:
                        nc.vector.tensor_copy(outsb, psO[s])
                    else:
                        nc.scalar.copy(outsb, psO[s])
                    outs.append(outsb)
                for s in range(2):
                    s8 = dti * 2 + s
                    psM = psGbc.tile([16, O], f32, tag="psG",
                                     name=f"psM{dti}_{s}")
                    nc.tensor.matmul(psM, meanmsb[:, s8, :], outs[s])
                    r0 = 2 * s8
                    rowsb = work.tile([16, O], f32, tag="rowsb",
                                      name=f"rowsb{dti}_{s}")
                    nc.vector.tensor_copy(rowsb, psM)
                    nc.sync.dma_start(out=out_d.ap()[r0:r0 + 2, :],
                                      in_=rowsb[r0:r0 + 2, :])


    nc.compile()
    return nc


def _host_inputs(x, gW1, gb1, gW2, gb2, W1, b1, W2, b2):
    """Per-core in_maps with all host-side layout transforms."""
    f = np.float32
    xt_full = np.ascontiguousarray(x.reshape(T, D).T.astype(f))       # [D, T]
    # W1 [E,D,H] -> [p, j, pair, s*64+h], e = 2*pair+s, d = 128*j+p
    w1sb = np.ascontiguousarray(
        W1.reshape(PAIRS, 2, 4, 128, H).transpose(3, 2, 0, 1, 4)
        .reshape(128, 4, PAIRS, 128).astype(f))
    # W2 [E,H,O] -> [s*64+h, pair, o], mean folded
    w2sb = np.ascontiguousarray(
        W2.reshape(PAIRS, 2, H, O).transpose(1, 2, 0, 3)
        .reshape(128, PAIRS, O).astype(f) / np.float32(FEW))
    b1sb = np.ascontiguousarray(
        b1.reshape(PAIRS, 2, H).transpose(1, 2, 0).reshape(128, PAIRS).astype(f))
    b2sb = np.ascontiguousarray(b2.astype(f) / np.float32(FEW))
    gw1f = gW1.reshape(4, 128, H).transpose(1, 0, 2).astype(f)  # [128,4,64]
    gw1hi = gw1f.astype(np.float16)
    gw1lo = ((gw1f - gw1hi.astype(f)) * 2048.0).astype(np.float16)
    gw1a = np.ascontiguousarray(
        np.stack([gw1hi, gw1lo], axis=2))                        # [128,4,2,64]
    gb1sb = np.ascontiguousarray(gb1.reshape(H, 1).astype(f))
    gw2a = np.ascontiguousarray(
        np.vstack([gW2.astype(f), gb2.reshape(1, E).astype(f)]))
    ident = np.eye(128, dtype=f)
    # maskp[e, pair, m] = 1 if e == 2*pair + m//64
    m = np.arange(128)
    pr = np.arange(PAIRS)
    ee = np.arange(E)
    maskp = (ee[:, None, None] == (2 * pr[None, :, None] + m[None, None, :] // 64)
             ).astype(f)
    # meanm[p, s, rrow] = 1 if rrow == 2*s + p//64
    ss = np.arange(NSLICE)
    rr = np.arange(E)
    meanm = (rr[None, None, :] == (2 * ss[None, :, None] + m[:, None, None] // 64)
             ).astype(f)

    h = np.float16
    shared = dict(w1=w1sb.astype(h), w2=w2sb.astype(h), b1=b1sb,
                  b2=b2sb.astype(h), gw1=gw1a, gb1=gb1sb, gw2a=gw2a,
                  ident=ident, maskp=maskp.astype(h), meanm=meanm.astype(h))
    in_maps = []
    for c in range(NCORES):
        im = dict(shared)
        xt_c = np.ascontiguousarray(xt_full[:, c * TLOC:(c + 1) * TLOC])
        xhi = xt_c.astype(h)
        im["xt16"] = xhi
        im["xlo"] = ((xt_c - xhi.astype(f)) * 2048.0).astype(h)
        in_maps.append(im)
    return in_maps


def kernel(x, gW1, gb1, gW2, gb2, W1, b1, W2, b2, _trace=False):
    from concourse.bass_utils import run_bass_kernel_spmd

    if "nc" not in _CACHE:
        _CACHE["nc"] = _build_nc()
    nc = _CACHE["nc"]
    args = [np.asarray(a, dtype=np.float32)
            for a in (x, gW1, gb1, gW2, gb2, W1, b1, W2, b2)]
    in_maps = _host_inputs(*args)
    try:
        kres = run_bass_kernel_spmd(nc, in_maps, core_ids=list(range(NCORES)),
                                    trace=_trace)
    except ModuleNotFoundError:
        # NTFF profile hook absent in this container; run without trace
        kres = run_bass_kernel_spmd(nc, in_maps, core_ids=list(range(NCORES)),
                                    trace=False)
    _CACHE["last_result"] = kres
    out = np.concatenate([kres.results[c]["out"] for c in range(NCORES)], axis=0)
    return out.reshape(B, 1, 1, O).astype(np.float32)
